# revision 20
# baseline (speedup 1.0000x reference)
"""Trainium2 Bass kernel for the 81-step LSTM decoder + masked softmax.

Math (per batch row b):
    z_t = x_t @ W_x + h_{t-1} @ W_h + b          (gates i, f, g, o; 100 each)
    i,f,o = sigmoid;  g = identity
    c_t = f*c_{t-1} + i*g;  h_t = o*c_t
    out_t = softmax(where(mask_t, h_t, -inf))

Strategy: data-parallel over batch (4096 -> 8 cores x 512), feature-major
("transposed") on-device layout: states are [100/101, 512] with hidden on
partitions and batch on the free dim.

Key tricks vs the straightforward version:
 - x @ W_x runs as fp8(e4m3, max 240) DoubleRow matmuls (2 contraction rows
   per cycle).  Accuracy is recovered with an error-feedback split
   x = x_hi + x_lo (both e4m3, x prescaled by 32 so everything stays in the
   e4m3 normal range) and optionally W = W8 + W8lo (e5m2 residual).  All
   chains accumulate into the same PSUM at a consistent global scale of 32,
   which is folded into the ACT input scales and host-side weight prescales
   (exact power-of-2 ops).
 - sigmoid(x) = (1 + tanh(x/2))/2.  tanh lives in the same activation table
   as exp, so the per-step sigmoid/exp mix causes zero table reloads.  The
   (1+t)/2 affines fold into scalar_tensor_tensor ops and host prescales:
       state S = 2*32*c, hist D = 4*32*h
       A  = (tf+1)*S;  B2 = (ti+1)*g;  S' = A*0.5 + B2;  D = (to+1)*S'
 - softmax: e^T = exp(D/(4*32) + mask_bias) via ACT bias column; the
   transpose back to batch-major is a plain bf16 matmul against an identity
   augmented with a ones column, so each transpose also emits the per-row
   masked sums for free.
 - outputs are staged in SBUF for 9 steps per batch tile and stored with one
   DMA each (3600B contiguous runs per row).
 - full-batch (512-wide) ops everywhere: PE per-instruction overhead
   (LDWEIGHTS + decode) dominates small matmuls, so fewer/bigger ops win.
   The softmax tail is lagged 2-4 steps behind the recurrence so the serial
   recurrence chain never stalls PE/ACT/DVE.
"""

import sys

if "/opt/trn_rl_repo" not in sys.path:
    sys.path.insert(0, "/opt/trn_rl_repo")

import numpy as np

P = 81       # places / timesteps
H = 100      # LSTM units
E = 512      # encoder feature width
B = 4096     # total batch
NCORES = 8
BS = B // NCORES          # 512 batch rows per core
NB = BS // 128            # 4 batch tiles of 128
NE = E // 128             # 4 feature chunks of 128
MASK_NEG = -1.0e5         # exp(h + MASK_NEG) == 0.0 exactly in fp32
KOUT = 3                  # output DMA batching window (81 % 3 == 0)
XS = 32.0                 # fp8 x prescale (exact power of 2; e4m3 max 240)
RH = 4                    # hist ring depth

_PROGRAM = None


def _build_program():
    import concourse.bacc as bacc
    import concourse.bass as bass
    import concourse.mybir as mybir
    from concourse.tile import TileContext
    from contextlib import ExitStack

    f32 = mybir.dt.float32
    bf16 = mybir.dt.bfloat16
    f8e4 = mybir.dt.float8e4
    f8e5 = mybir.dt.float8e5
    TANH = mybir.ActivationFunctionType.Tanh
    EXP = mybir.ActivationFunctionType.Exp
    CPY = mybir.ActivationFunctionType.Copy
    ADD = mybir.AluOpType.add
    MUL = mybir.AluOpType.mult
    DR = mybir.MatmulPerfMode.DoubleRow

    nc = bacc.Bacc(None, target_bir_lowering=False)

    # ---- DRAM tensors -------------------------------------------------
    # x hi/lo fp8 split, transposed, prescaled by XS:
    #   x8[t, c*4+ec, p, b] = chain_c(XS*x)[b, t, ec*128+p], c in {hi, lo}
    # The tanh gates (i, f, o) use the hi chain only (tanh squashes the fp8
    # quantization error); the identity gate g uses hi+lo plus an e5m2
    # weight-residual chain for near-bf16 accuracy.
    x8_d = nc.dram_tensor("x8", [P, 2 * NE, 128, BS], f8e4, kind="ExternalInput")
    w8_d = nc.dram_tensor("w8", [128, NE, 400], f8e4, kind="ExternalInput")
    w8lo_d = nc.dram_tensor("w8lo", [128, NE, 400], f8e5, kind="ExternalInput")
    # whb: [0:400] W_h(+bias row), [400:501] identity+ones col for transposes
    whb_d = nc.dram_tensor("whb", [128, 504], bf16, kind="ExternalInput")
    csb_d = nc.dram_tensor("csb", [128, P], f32, kind="ExternalInput")
    h0T_d = nc.dram_tensor("h0T", [101, BS], bf16, kind="ExternalInput")
    out_d = nc.dram_tensor("out", [BS, P, H], f32, kind="ExternalOutput")

    with ExitStack() as ctx:
        tc = ctx.enter_context(TileContext(nc))
        consts = ctx.enter_context(tc.tile_pool(name="consts", bufs=1))
        xpool = ctx.enter_context(tc.tile_pool(name="xpool", bufs=10))
        tpool = ctx.enter_context(tc.tile_pool(name="tpool", bufs=6))
        abpool = ctx.enter_context(tc.tile_pool(name="abpool", bufs=6))
        epool = ctx.enter_context(tc.tile_pool(name="epool", bufs=3))
        rpool = ctx.enter_context(tc.tile_pool(name="rpool", bufs=3))
        opool = ctx.enter_context(tc.tile_pool(name="opool", bufs=8))
        zpool = ctx.enter_context(tc.tile_pool(name="zpool", bufs=6, space="PSUM"))
        etpool = ctx.enter_context(tc.tile_pool(name="etpool", bufs=2, space="PSUM"))

        # ---- prefetch weights + x(0) first (startup critical path) ----
        xt_pre = {}
        w8 = consts.tile([128, NE, 400], f8e4)
        nc.sync.dma_start(out=w8, in_=w8_d[:, :, :])
        w8lo = consts.tile([128, NE, 400], f8e5)
        nc.sync.dma_start(out=w8lo, in_=w8lo_d[:, :, :])

        def dma_x(t):
            x8t = xpool.tile([128, 2 * NE, BS], f8e4, name=f"x8_{t}", tag="x8")
            nc.sync.dma_start(out=x8t, in_=x8_d[t].rearrange("k p b -> p k b"))
            return x8t

        for t0 in range(2):
            xt_pre[t0] = dma_x(t0)

        # ---- constants ------------------------------------------------
        whb = consts.tile([128, 504], bf16)
        nc.sync.dma_start(out=whb, in_=whb_d[:, :])
        csb = consts.tile([128, P], f32)
        nc.sync.dma_start(out=csb, in_=csb_d[:, :])
        mb = csb[0:H, 0:P]
        idn = whb[0:H, 400:501]

        # hist ring (D = 4*XS*h on rows 0:100, ones row at 100)
        hist = [consts.tile([H + 1, BS], bf16, name=f"hist{j}") for j in range(RH)]
        for j in range(RH - 1):
            nc.sync.dma_start(out=hist[j][H : H + 1, :], in_=h0T_d[H : H + 1, :])
        nc.sync.dma_start(out=hist[RH - 1], in_=h0T_d[:, :])
        S = consts.tile([H, BS], f32)  # S = 2*XS*c
        nc.vector.memset(S, 0.0)

        # lag-managed intermediates
        etp_hist = {}   # t -> etp psum tile
        r4_hist = {}    # t -> r4 tile
        ot_hist = {}    # (win, j) -> out staging tile
        e_hist = {}     # t -> e^T tile

        def emit_exp(t):
            et = epool.tile([H, BS], bf16, name=f"e_{t}", tag="e")
            nc.scalar.activation(
                et, hist[t % RH][0:H, :], EXP,
                scale=0.25 / XS, bias=mb[:, t : t + 1],
            )
            e_hist[t] = et

        def emit_transposes(t):
            # batch-major e + masked row sums via one bf16 matmul per tile:
            # out[b, :] = e_sliceT @ [I | 1]  (last col = per-row sum)
            et = e_hist.pop(t)
            etp = etpool.tile([128, NB, 104], f32, name=f"etp_{t}", tag="etp")
            for j in range(NB):
                nc.tensor.matmul(
                    etp[:, j, 0:101],
                    et[:, 128 * j : 128 * (j + 1)],
                    idn,
                    start=True, stop=True,
                )
            etp_hist[t] = etp

        def emit_recip(t):
            etp = etp_hist[t]
            r4 = rpool.tile([128, NB, 1], f32, name=f"r_{t}", tag="r")
            nc.vector.reciprocal(r4, etp[:, :, 100:101])
            r4_hist[t] = r4

        def emit_finals(t, lo, hi, engine):
            # out_bt = e_bt * r_bt on ACT (Copy w/ scale column) or DVE.
            etp = etp_hist[t]
            r4 = r4_hist[t]
            win = t // KOUT
            for j in range(lo, hi):
                if (win, j) not in ot_hist:
                    ot_hist[(win, j)] = opool.tile(
                        [128, KOUT, H], f32, name=f"ot_{win}_{j}", tag="ot"
                    )
                dst = ot_hist[(win, j)][:, t % KOUT, :]
                if engine == "act":
                    nc.scalar.activation(dst, etp[:, j, 0:H], CPY,
                                         scale=r4[:, j, :])
                else:
                    nc.vector.tensor_scalar_mul(dst, etp[:, j, 0:H], r4[:, j, :])

        def emit_store(t):
            etp_hist.pop(t, None)
            r4_hist.pop(t, None)
            if t % KOUT == KOUT - 1:
                win = t // KOUT
                t0 = t - KOUT + 1
                for j in range(NB):
                    ot = ot_hist.pop((win, j))
                    nc.gpsimd.dma_start(
                        out=out_d[128 * j : 128 * (j + 1), t0 : t + 1, :], in_=ot
                    )

        def step(t):
            # ---- stream x_t -------------------------------------------
            if t in xt_pre:
                x8t = xt_pre.pop(t)
            else:
                x8t = dma_x(t)

            # ---- z accumulation (PE): full batch, one PSUM bank/slot --
            # slot order [i, f, o, g] (host packs W columns accordingly)
            # i,f,o: fp8 DoubleRow, hi chain only (2 mms, K=256 each)
            # g:     hi@W8 + lo@W8 + hi@W8lo (6 mms) ~ bf16 accuracy
            zs = []
            for slot in range(4):
                wc = slice(slot * 100, (slot + 1) * 100)
                z = zpool.tile([H, BS], f32, name=f"z_{t}_{slot}", tag="z")
                chains = ((w8, 0),) if slot < 3 else                     ((w8, 0), (w8, NE), (w8lo, 0))
                first = True
                for wt, xi in chains:
                    for ep in (0, 2):
                        nc.tensor.matmul(
                            z, wt[:, ep : ep + 2, wc],
                            x8t[:, xi + ep : xi + ep + 2, :],
                            start=first, stop=False, perf_mode=DR,
                        )
                        first = False
                nc.tensor.matmul(
                    z, whb[0 : H + 1, wc], hist[(t - 1) % RH],
                    start=False, stop=True,
                )
                zs.append(z)

            # PE: lagged transposes (deps ready long ago -> no PE stall)
            if t - 3 >= 0:
                emit_transposes(t - 3)

            # DVE: lagged reciprocal first (deps ready -> fills DVE idle)
            if t - 4 >= 0:
                emit_recip(t - 4)

            # ---- gates + cell (full batch) ----------------------------
            # ACT order: ti, tf, to, exp(t-2); DVE: B2, A, S, D
            ti = tpool.tile([H, BS], f32, name=f"ti_{t}", tag="ti")
            nc.scalar.activation(ti, zs[0], TANH, scale=1.0 / XS)
            tf = tpool.tile([H, BS], f32, name=f"tf_{t}", tag="tf")
            nc.scalar.activation(tf, zs[1], TANH, scale=1.0 / XS)

            b2 = abpool.tile([H, BS], f32, name=f"b2_{t}", tag="b2")
            nc.vector.scalar_tensor_tensor(b2, ti, 1.0, zs[3], ADD, MUL)

            to = tpool.tile([H, BS], f32, name=f"to_{t}", tag="to")
            nc.scalar.activation(to, zs[2], TANH, scale=1.0 / XS)
            if t - 2 >= 0:
                emit_exp(t - 2)

            a = abpool.tile([H, BS], f32, name=f"a_{t}", tag="a")
            nc.vector.scalar_tensor_tensor(a, tf, 1.0, S, ADD, MUL)
            nc.vector.scalar_tensor_tensor(S, a, 0.5, b2, MUL, ADD)
            nc.vector.scalar_tensor_tensor(
                hist[t % RH][0:H, :], to, 1.0, S, ADD, MUL
            )

            if t - 4 >= 0:
                emit_finals(t - 4, 3, 4, "dve")
                emit_finals(t - 4, 0, 3, "act")
                emit_store(t - 4)

        for t in range(P):
            step(t)

        # ---- drain the lag pipeline ----------------------------------
        for t in range(P - 2, P):
            emit_exp(t)
        for t in range(P - 3, P):
            emit_transposes(t)
        for t in range(P - 4, P):
            emit_recip(t)
            emit_finals(t, 0, 3, "act")
            emit_finals(t, 3, NB, "dve")
            emit_store(t)

    nc.compile()
    return nc


def _get_program():
    global _PROGRAM
    if _PROGRAM is None:
        _PROGRAM = _build_program()
    return _PROGRAM


def _prep_in_maps(h_enc, h0, W_x, W_h, b, mask):
    import ml_dtypes

    bf16 = ml_dtypes.bfloat16
    e4 = ml_dtypes.float8_e4m3
    e5 = ml_dtypes.float8_e5m2

    h_enc = np.asarray(h_enc, dtype=np.float32)
    h0 = np.asarray(h0, dtype=np.float32)
    W_x = np.asarray(W_x, dtype=np.float32)
    W_h = np.asarray(W_h, dtype=np.float32)
    b = np.asarray(b, dtype=np.float32)
    mask = np.asarray(mask)

    # column reorder [i, f, o, g] + tanh prescale (i,f,o cols x0.5)
    perm = np.concatenate([np.arange(0, 200), np.arange(300, 400),
                           np.arange(200, 300)])
    sc = np.ones(400, np.float32)
    sc[0:300] = 0.5  # post-perm: i,f,o are the first 300 cols
    Wxp = W_x[:, perm] * sc[None, :]
    Whp = W_h[:, perm] * sc[None, :]
    bp = b[perm] * sc

    # device lhsT layout [128, ec, 400]
    W8f = Wxp.astype(e4)
    W8lof = (Wxp - W8f.astype(np.float32)).astype(e5)
    w8 = np.ascontiguousarray(W8f.reshape(NE, 128, 400).transpose(1, 0, 2))
    w8lo = np.ascontiguousarray(
        W8lof.reshape(NE, 128, 400).transpose(1, 0, 2))

    whb = np.zeros((128, 504), np.float32)
    whb[0:H, 0:400] = Whp * 0.25   # hist carries D = 4*XS*h
    whb[H, 0:400] = bp * XS        # ones row adds XS*b
    whb[0:H, 400:500] = np.eye(100, dtype=np.float32)
    whb[0:H, 500] = 1.0
    whb = whb.astype(bf16)

    csb = np.zeros((128, P), np.float32)
    csb[0:H, 0:P] = np.where(mask, 0.0, MASK_NEG).astype(np.float32).T

    in_maps = []
    for c in range(NCORES):
        shard = h_enc[c * BS : (c + 1) * BS]           # [BS, P, E]
        xs = (XS * shard).transpose(1, 2, 0).reshape(P, NE, 128, BS)
        xhi = xs.astype(e4)
        xlo = (xs - xhi.astype(np.float32)).astype(e4)
        x8 = np.empty((P, 2 * NE, 128, BS), e4)
        x8[:, 0:NE] = xhi
        x8[:, NE:] = xlo

        h0T = np.zeros((101, BS), np.float32)
        h0T[0:H] = (4.0 * XS) * h0[c * BS : (c + 1) * BS].T
        h0T[H] = 1.0
        in_maps.append({
            "x8": np.ascontiguousarray(x8), "w8": w8, "w8lo": w8lo,
            "whb": whb, "csb": csb, "h0T": h0T.astype(bf16),
        })
    return in_maps


def run(inputs: dict, trace: bool = False):
    """Run on 8 cores; returns (full_output, exec_time_ns_or_None)."""
    from concourse.bass_utils import run_bass_kernel_spmd

    nc = _get_program()
    in_maps = _prep_in_maps(**inputs)
    res = run_bass_kernel_spmd(
        nc, in_maps, core_ids=list(range(NCORES)), trace=trace
    )
    out = np.concatenate([r["out"] for r in res.results], axis=0)
    return out, res.exec_time_ns


def kernel(**inputs) -> np.ndarray:
    out, _ = run(inputs, trace=False)
    return out
